# revision 1
# baseline (speedup 1.0000x reference)
"""Trainium2 Bass kernel for nn_ArbitraryRNN (4-layer masked Elman RNN).

kernel(**inputs) takes the FULL inputs (x [2048,64,256] plus 256x256
weights/masks), runs a distributed Bass kernel on 8 NeuronCores, and
returns the full [64,256] output (last timestep of layer2 + skip sums).

Strategy: data-parallel over batch (8 cores x B=8; weights replicated).
Each core runs all four recurrences ("lanes": L0, L1, L2, Ls=skip)
chunk-pipelined as a wavefront: in round r, L0 processes chunk r, L1 and
Ls chunk r-1, L2 chunk r-2 (the sequence dim cannot be sharded, so the
four serial tanh chains are interleaved to keep the engines busy).

Per chunk (C=64 steps) the input transform xg = wihT.T @ h_prev plus the
bias (a rank-1 ones-matmul) is bulk-matmul'd straight into PSUM, one
PSUM bank per output half (start=True clears has_written for exactly
that bank). The per-step recurrent matmuls accumulate on top
(start=False) and a single ScalarE tanh per lane-step reads PSUM and
writes the new hidden state (H-major layout [128, kh, C, B]) into SBUF,
where it directly feeds both the next step's matmul rhs and the
consumer lane's bulk rhs. Host pre-transposes x/weights and pre-applies
the masks; the final h2+hs add runs on VectorE.
"""

import numpy as np

T, B_TOTAL, H = 2048, 64, 256
N_CORES = 8
B = B_TOTAL // N_CORES  # 8
C = 64                  # chunk length: C*B*4B = one 2KB PSUM bank per m-half
KH = MH = 2

LANES = [0, 1, 2, 3]  # L0, L1, L2, Ls
LAG = {0: 0, 1: 1, 2: 2, 3: 1}
PROD = {1: 0, 2: 1, 3: 0}
CONS = {0: [1, 3], 1: [2], 2: [], 3: []}


def _build(dt):
    import concourse.bass as bass
    import concourse.mybir as mybir

    F32 = mybir.dt.float32
    R = T // C
    TOTAL_ROUNDS = R + 2
    assert C * B == 512

    nc = bass.Bass()

    xT = nc.declare_dram_parameter("xT", [2, 128, T, B], dt, isOutput=False)
    whhT = nc.declare_dram_parameter("whhT", [4, 256, 256], dt, isOutput=False)
    wihT = nc.declare_dram_parameter("wihT", [4, 256, 256], dt, isOutput=False)
    biasP = nc.declare_dram_parameter("bias", [1, 4, 256], F32, isOutput=False)
    outP = nc.declare_dram_parameter("out", [2, 128, B], F32, isOutput=True)

    cms = []

    def ent(cm):
        cms.append(cm)
        return cm.__enter__()

    whh_sb = ent(nc.sbuf_tensor("whh_sb", [128, 4, KH, MH, 128], dt))
    wih_sb = ent(nc.sbuf_tensor("wih_sb", [128, 4, KH, MH, 128], dt))
    bias_sb = ent(nc.sbuf_tensor("bias_sb", [128, 4, MH, 128], F32))
    ones_sb = ent(nc.sbuf_tensor("ones_sb", [128, C * B], F32))
    hzero = ent(nc.sbuf_tensor("hzero", [128, KH, B], dt))
    x_sb = ent(nc.sbuf_tensor("x_sb", [128, 2, KH, C, B], dt))
    h_sb = [ent(nc.sbuf_tensor(f"h{l}_sb", [128, 2, KH, C, B], dt)) for l in LANES]
    out_sb = ent(nc.sbuf_tensor("out_sb", [128, MH, B], F32))

    psum = [ent(nc.psum_tensor(f"ps{l}", [128, MH, C, B], F32)) for l in LANES]

    s_h = [ent(nc.semaphore(f"s_h{l}")) for l in LANES]
    s_mm = [ent(nc.semaphore(f"s_mm{l}")) for l in LANES]
    s_blk = [ent(nc.semaphore(f"s_blk{l}")) for l in LANES]
    s_xdma = ent(nc.semaphore("s_xdma"))
    s_init = ent(nc.semaphore("s_init"))
    s_fin = ent(nc.semaphore("s_fin"))
    s_out = ent(nc.semaphore("s_out"))

    block = ent(nc.Block())

    def chunk_of(lane, r):
        return r - LAG[lane]

    def active(lane, r):
        return 0 <= chunk_of(lane, r) < R

    n_init = {"n": 0}

    @block.sync
    def _(sync):
        for l in range(4):
            for k in range(KH):
                for m in range(MH):
                    sync.dma_start(
                        out=whh_sb[:, l, k, m, :],
                        in_=whhT.ap()[l, k * 128 : (k + 1) * 128, m * 128 : (m + 1) * 128],
                    ).then_inc(s_init, 16)
                    sync.dma_start(
                        out=wih_sb[:, l, k, m, :],
                        in_=wihT.ap()[l, k * 128 : (k + 1) * 128, m * 128 : (m + 1) * 128],
                    ).then_inc(s_init, 16)
                    n_init["n"] += 2
            sync.dma_start(
                out=bias_sb[:1, l, :, :],
                in_=biasP.ap()[0:1, l, :].rearrange("o (mh ml) -> o mh ml", ml=128),
            ).then_inc(s_init, 16)
            n_init["n"] += 1
        for c in range(R):
            if c >= 2:
                sync.wait_ge(s_blk[0], c - 1)
            sync.dma_start(
                out=x_sb[:, c % 2, :, :, :],
                in_=xT.ap()[:, :, c * C : (c + 1) * C, :].rearrange(
                    "kh kl t b -> kl kh t b"
                ),
            ).then_inc(s_xdma, 16)
        sync.wait_ge(s_fin, 1)
        sync.dma_start(
            out=outP.ap().rearrange("mh ml b -> ml mh b"), in_=out_sb[:, :, :]
        ).then_inc(s_out, 16)
        sync.wait_ge(s_out, 16)

    @block.gpsimd
    def _(gpsimd):
        gpsimd.memset(ones_sb[:1, :], 1.0).then_inc(s_init, 1)
        gpsimd.memset(hzero[:, :, :], 0.0).then_inc(s_init, 1)

    INIT_THRESH = n_init["n"] * 16 + 2

    @block.tensor
    def _(pe):
        pe.wait_ge(s_init, INIT_THRESH)

        def emit_bulk(lane, c):
            last = None
            for m in range(MH):
                outap = psum[lane][:, m, :, :]
                for k in range(KH):
                    if lane == 0:
                        rhs = x_sb[:, c % 2, k, :, :]
                    else:
                        rhs = h_sb[PROD[lane]][:, c % 2, k, :, :]
                    pe.matmul(
                        outap,
                        wih_sb[:, lane, k, m, :],
                        rhs,
                        start=(k == 0),
                        stop=False,
                        skip_group_check=True,
                    )
                last = pe.matmul(
                    outap,
                    bias_sb[:1, lane, m, :],
                    ones_sb[:1, :],
                    start=False,
                    stop=False,
                    skip_group_check=True,
                )
            last.then_inc(s_blk[lane], 1)

        def emit_rec_step(lane, c, t):
            n = c * C + t
            if n >= 1:
                pe.wait_ge(s_h[lane], n)
            ins = None
            for m in range(MH):
                outap = psum[lane][:, m, t, :]
                for k in range(KH):
                    if t == 0:
                        if c == 0:
                            rhs = hzero[:, k, :]
                        else:
                            rhs = h_sb[lane][:, (c - 1) % 2, k, C - 1, :]
                    else:
                        rhs = h_sb[lane][:, c % 2, k, t - 1, :]
                    ins = pe.matmul(
                        outap,
                        whh_sb[:, lane, k, m, :],
                        rhs,
                        start=False,
                        stop=(m == MH - 1 and k == KH - 1),
                        skip_group_check=True,
                    )
            ins.then_inc(s_mm[lane], 1)

        for r in range(TOTAL_ROUNDS):
            lanes_now = [l for l in LANES if active(l, r)]
            for lane in lanes_now:
                c = chunk_of(lane, r)
                if c >= 1:
                    pe.wait_ge(s_h[lane], c * C)
                if lane == 0:
                    pe.wait_ge(s_xdma, 16 * (c + 1))
                else:
                    pe.wait_ge(s_h[PROD[lane]], (c + 1) * C)
                emit_bulk(lane, c)
            for t in range(C):
                for lane in lanes_now:
                    emit_rec_step(lane, chunk_of(lane, r), t)

    @block.scalar
    def _(scalar):
        import concourse.mybir as mybir

        for r in range(TOTAL_ROUNDS):
            lanes_now = [l for l in LANES if active(l, r)]
            for t in range(C):
                for lane in lanes_now:
                    c = chunk_of(lane, r)
                    n = c * C + t
                    if t == 0 and c >= 2:
                        for cons in CONS[lane]:
                            scalar.wait_ge(s_blk[cons], c - 1)
                    scalar.wait_ge(s_mm[lane], n + 1)
                    scalar.activation(
                        h_sb[lane][:, c % 2, :, t, :],
                        psum[lane][:, :, t, :],
                        mybir.ActivationFunctionType.Tanh,
                    ).then_inc(s_h[lane], 1)

    @block.vector
    def _(vector):
        R_ = T // C
        vector.wait_ge(s_h[2], T)
        vector.wait_ge(s_h[3], T)
        h2 = h_sb[2][:, (R_ - 1) % 2, :, C - 1, :]
        hs = h_sb[3][:, (R_ - 1) % 2, :, C - 1, :]
        vector.tensor_add(out_sb[:, :, :], h2, hs).then_inc(s_fin, 1)

    for cm in reversed(cms):
        cm.__exit__(None, None, None)
    return nc


def _prep_inputs(inputs, dt_np):
    x = np.asarray(inputs["x"], dtype=np.float32)
    names = ["0", "1", "2", "s"]
    whhT = np.stack([np.asarray(inputs[f"w_hh{n}"], dtype=np.float32).T for n in names])
    masks = [
        None,
        np.asarray(inputs["mask1"]),
        np.asarray(inputs["mask2"]),
        np.asarray(inputs["mask_skip"]),
    ]
    wihT_l = []
    for li, n in enumerate(names):
        w = np.asarray(inputs[f"w_ih{n}"], dtype=np.float32)
        if masks[li] is not None:
            w = w * masks[li].astype(np.float32)
        wihT_l.append(w.T)
    wihT = np.stack(wihT_l)
    bias = np.stack(
        [
            np.asarray(inputs[f"b_ih{n}"], dtype=np.float32)
            + np.asarray(inputs[f"b_hh{n}"], dtype=np.float32)
            for n in names
        ]
    )[None]

    whhT = whhT.astype(dt_np)
    wihT = wihT.astype(dt_np)
    bias = bias.astype(np.float32)

    in_maps = []
    for g in range(N_CORES):
        xg = x[:, g * B : (g + 1) * B, :]
        xTg = np.ascontiguousarray(xg.transpose(2, 0, 1).reshape(2, 128, T, B)).astype(
            dt_np
        )
        in_maps.append({"xT": xTg, "whhT": whhT, "wihT": wihT, "bias": bias})
    return in_maps


_CACHE = {}


def kernel(**inputs) -> np.ndarray:
    import concourse.mybir as mybir
    from concourse.bass_utils import run_bass_kernel_spmd

    dt = mybir.dt.float32
    dt_np = np.float32

    if "nc" not in _CACHE:
        _CACHE["nc"] = _build(dt)
    nc = _CACHE["nc"]

    in_maps = _prep_inputs(inputs, dt_np)
    res = run_bass_kernel_spmd(nc, in_maps, core_ids=list(range(N_CORES)))

    outs = []
    for g in range(N_CORES):
        o = np.asarray(res.results[g]["out"], dtype=np.float32)  # [2, 128, B]
        outs.append(o.reshape(H, B).T)
    return np.concatenate(outs, axis=0).astype(np.float32)
